# revision 1
# baseline (speedup 1.0000x reference)
"""Trainium2 Bass kernel for nn_Block_26628797235524 (Mamba-style cross-scan SSM block).

Sharding: batch B=8 -> one batch element per NeuronCore (8 cores, SPMD, no
collectives). Each core runs the full block for its batch element:
  in_proj -> conv(1x1x1)+silu -> dual-order selective scan (K=2, DIN=256,
  DST=16) -> combine -> layernorm -> gate -> out_proj.

Layout on chip: channel-major (128-partition d tiles, time on the free dim).
The sequential scan runs as `tensor_tensor_scan` (state = dA*state + dBu)
over 128 channels x 1024-step slabs, chained via the previous slab's last
column; 16 state dims (n) are handled as separate scan lanes.

kernel(**inputs) takes the FULL unsharded inputs and returns the FULL output.
"""

import os
import sys
from contextlib import ExitStack

import numpy as np

_RL = "/opt/trn_rl_repo"
if os.path.isdir(_RL) and _RL not in sys.path:
    sys.path.insert(0, _RL)

import concourse.bass as bass
import concourse.bacc as bacc
import concourse.tile as tile
from concourse import mybir
from concourse.bass_utils import run_bass_kernel_spmd

# Problem sizes (hardcoded per the task contract).
B, T, H, W, DIM = 8, 16, 16, 16, 128
DIN, DST, DTR, KG = 256, 16, 8, 2
L = T * H * W          # 4096
P = 128                # partitions
DH = DIN // P          # 2 d-half tiles per direction
LC = 1024              # scan slab length
NSLAB = L // LC        # 4
NCORES = 8

F32 = mybir.dt.float32
AF = mybir.ActivationFunctionType
ALU = mybir.AluOpType
MM_F = 512             # matmul free-dim chunk (one PSUM bank)
NMM = L // MM_F        # 8 chunks over L


def _declare_drams(nc):
    d = {}

    def inp(name, shape):
        d[name] = nc.dram_tensor(name, list(shape), F32, kind="ExternalInput")

    inp("xT", (P, L))                  # per-core batch slice, (DIM, L) channel-major
    inp("w_in", (P, 4 * P))            # in_proj_w.T  (128, 512)
    inp("conv_sc", (DH, P, 1))         # conv_w per d-half column
    inp("conv_bi", (DH, P, 1))         # conv_b
    inp("w_xproj", (KG, DH, P, 40))    # x_proj_w[k].T in 2 pi-chunks
    inp("w_dt", (KG, DTR, DIN))        # dt_w[k].T  (8, 256)
    inp("dt_bias", (KG, DH, P, 1))
    inp("a_mat", (KG, DH, P, DST))     # A = -exp(A_logs)
    inp("ds_vec", (KG, DH, P, 1))
    inp("lnw", (DH, P, 1))
    inp("lnb", (DH, P, 1))
    inp("w_out", (DH, P, P))           # out_proj_w.T in 2 pi-chunks
    inp("ident", (P, P))               # identity: PE copy/accumulate matmuls
    d["xs_dram"] = nc.dram_tensor("xs_dram", [KG, DH, P, L], F32)  # spilled xs
    d["bc_dram"] = nc.dram_tensor("bc_dram", [KG, 2, DST, L], F32)  # B/C rows for bcast
    d["z_dram"] = nc.dram_tensor("z_dram", [DH, P, L], F32)      # spilled silu(z)
    d["y0_dram"] = nc.dram_tensor("y0_dram", [DH, P, L], F32)    # spilled y_fwd (natural order)
    d["outT"] = nc.dram_tensor("outT", [P, L], F32, kind="ExternalOutput")
    return d


def _body(tc, d):
    nc = tc.nc
    with ExitStack() as ctx:
        const = ctx.enter_context(tc.tile_pool(name="const", bufs=1))

        # ---- constants ----
        w_in = const.tile([P, 4 * P], F32, tag="w_in", name="w_in")
        nc.sync.dma_start(w_in[:], d["w_in"][:])
        conv_sc = [const.tile([P, 1], F32, tag=f"csc{i}", name=f"csc{i}") for i in range(DH)]
        conv_bi = [const.tile([P, 1], F32, tag=f"cbi{i}", name=f"cbi{i}") for i in range(DH)]
        for i in range(DH):
            nc.sync.dma_start(conv_sc[i][:], d["conv_sc"][i])
            nc.sync.dma_start(conv_bi[i][:], d["conv_bi"][i])
        w_xproj = [[const.tile([P, 40], F32, tag=f"wxp{k}{i}", name=f"wxp{k}{i}") for i in range(DH)]
                   for k in range(KG)]
        w_dt = [const.tile([DTR, DIN], F32, tag=f"wdt{k}", name=f"wdt{k}") for k in range(KG)]
        dt_bias = [[const.tile([P, 1], F32, tag=f"dtb{k}{i}", name=f"dtb{k}{i}") for i in range(DH)]
                   for k in range(KG)]
        a_mat = [[const.tile([P, DST], F32, tag=f"am{k}{i}", name=f"am{k}{i}") for i in range(DH)]
                 for k in range(KG)]
        ds_vec = [[const.tile([P, 1], F32, tag=f"dsv{k}{i}", name=f"dsv{k}{i}") for i in range(DH)]
                  for k in range(KG)]
        for k in range(KG):
            nc.sync.dma_start(w_dt[k][:], d["w_dt"][k])
            for i in range(DH):
                nc.sync.dma_start(w_xproj[k][i][:], d["w_xproj"][k, i])
                nc.sync.dma_start(dt_bias[k][i][:], d["dt_bias"][k, i])
                nc.sync.dma_start(a_mat[k][i][:], d["a_mat"][k, i])
                nc.sync.dma_start(ds_vec[k][i][:], d["ds_vec"][k, i])
        lnw = [const.tile([P, 1], F32, tag=f"lnw{i}", name=f"lnw{i}") for i in range(DH)]
        lnb = [const.tile([P, 1], F32, tag=f"lnb{i}", name=f"lnb{i}") for i in range(DH)]
        w_out = [const.tile([P, P], F32, tag=f"wo{i}", name=f"wo{i}") for i in range(DH)]
        for i in range(DH):
            nc.sync.dma_start(lnw[i][:], d["lnw"][i])
            nc.sync.dma_start(lnb[i][:], d["lnb"][i])
            nc.sync.dma_start(w_out[i][:], d["w_out"][i])
        ones_col = const.tile([P, 1], F32, tag="ones_col", name="ones_col")
        nc.vector.memset(ones_col[:], 1.0)
        one_b = const.tile([P, 1], F32, tag="one_b", name="one_b")
        nc.vector.memset(one_b[:], 1.0)
        ones_row = const.tile([1, P], F32, tag="ones_row", name="ones_row")
        nc.vector.memset(ones_row[:], 1.0)
        ident = const.tile([P, P], F32, tag="ident", name="ident")
        nc.sync.dma_start(ident[:], d["ident"][:])

        y1pool = ctx.enter_context(tc.tile_pool(name="y1pool", bufs=1))

        # ========== Phase 1: in_proj + conv/silu; Phase 2: scan orderings ======
        with tc.tile_pool(name="p1", bufs=1) as p1pool, \
             tc.tile_pool(name="p1ps", bufs=4, space=bass.MemorySpace.PSUM) as p1ps:
            xT = p1pool.tile([P, L], F32, tag="xT", name="xT")
            nc.sync.dma_start(xT[:], d["xT"][:])
            xh_s = [p1pool.tile([P, L], F32, tag=f"xh{i}", name=f"xh{i}") for i in range(DH)]
            z_t = p1pool.tile([P, L], F32, tag="z_t", name="z_t")
            for po in range(4):
                for c in range(NMM):
                    pt = p1ps.tile([P, MM_F], F32, tag="mm", name="mm")
                    nc.tensor.matmul(
                        pt[:], w_in[:, po * P:(po + 1) * P],
                        xT[:, c * MM_F:(c + 1) * MM_F], start=True, stop=True)
                    if po < DH:  # xh rows: silu(v), v = xh*conv_w + conv_b
                        sg = p1pool.tile([P, MM_F], F32, tag="sg", name="sg", bufs=3)
                        nc.scalar.activation(sg[:], pt[:], AF.Sigmoid,
                                             bias=conv_bi[po][:],
                                             scale=conv_sc[po][:])
                        vv = p1pool.tile([P, MM_F], F32, tag="vv", name="vv", bufs=3)
                        nc.vector.tensor_scalar(vv[:], pt[:], conv_sc[po][:],
                                                conv_bi[po][:], ALU.mult, ALU.add)
                        nc.vector.tensor_tensor(
                            xh_s[po][:, c * MM_F:(c + 1) * MM_F], vv[:], sg[:],
                            ALU.mult)
                    else:        # z rows: silu(z) -> spill to DRAM
                        sg = p1pool.tile([P, MM_F], F32, tag="sg", name="sg", bufs=3)
                        nc.scalar.activation(sg[:], pt[:], AF.Sigmoid)
                        nc.vector.tensor_tensor(
                            z_t[:, c * MM_F:(c + 1) * MM_F], pt[:], sg[:], ALU.mult)
                if po >= DH:
                    nc.sync.dma_start(d["z_dram"][po - DH], z_t[:])

            # xs[k=0]: (h,w,t) reorder; xs[k=1]: reversed natural. Both -> DRAM.
            for i in range(DH):
                ord0 = p1pool.tile([P, L], F32, tag="ord0", name="ord0")
                src = xh_s[i][:].rearrange("p (t hw) -> p hw t", t=T, hw=H * W)
                dst = ord0[:].rearrange("p (hw t) -> p hw t", hw=H * W, t=T)
                nc.vector.tensor_copy(dst, src)
                nc.sync.dma_start(d["xs_dram"][0, i], ord0[:])
                rev = p1pool.tile([P, L], F32, tag="rev", name="rev")
                nc.vector.tensor_copy(rev[:], xh_s[i][:, ::-1])
                nc.sync.dma_start(d["xs_dram"][1, i], rev[:])

        # ================= Per-direction pipeline =================
        y1_tiles = []
        for k in range(KG):
            with tc.tile_pool(name=f"kp{k}", bufs=1) as kpool:
                delta = [kpool.tile([P, L], F32, tag=f"delta{i}", name=f"delta{i}")
                         for i in range(DH)]
                # ---- Phase 3: x_dbl (dts-in -> SBUF; B/C rows -> DRAM) ----
                with tc.tile_pool(name=f"kd{k}", bufs=1) as kdpool, \
                     tc.tile_pool(name=f"kps{k}", bufs=4,
                                  space=bass.MemorySpace.PSUM) as kps:
                    xs_d = [kdpool.tile([P, L], F32, tag=f"xsd{i}", name=f"xsd{i}")
                            for i in range(DH)]
                    for i in range(DH):
                        nc.sync.dma_start(xs_d[i][:], d["xs_dram"][k, i])
                    xdbl_d = kdpool.tile([DTR, L], F32, tag="xdbl_d", name="xdbl_d")
                    parts = [(0, DTR, None), (DTR, DST, 0), (DTR + DST, DST, 1)]
                    for c in range(NMM):
                        for row0, nrow, bc in parts:
                            pt = kps.tile([DST, MM_F], F32, tag="mmx", name="mmx")
                            nc.tensor.matmul(pt[:nrow, :],
                                             w_xproj[k][0][:, row0:row0 + nrow],
                                             xs_d[0][:, c * MM_F:(c + 1) * MM_F],
                                             start=True, stop=False)
                            nc.tensor.matmul(pt[:nrow, :],
                                             w_xproj[k][1][:, row0:row0 + nrow],
                                             xs_d[1][:, c * MM_F:(c + 1) * MM_F],
                                             start=False, stop=True)
                            if bc is None:
                                nc.scalar.activation(
                                    xdbl_d[:, c * MM_F:(c + 1) * MM_F],
                                    pt[:nrow, :], AF.Copy)
                            else:
                                bcs = kdpool.tile([DST, MM_F], F32, tag="bcs",
                                                  name="bcs", bufs=2)
                                nc.scalar.activation(bcs[:], pt[:nrow, :], AF.Copy)
                                nc.sync.dma_start(
                                    d["bc_dram"][k, bc, :,
                                                 c * MM_F:(c + 1) * MM_F],
                                    bcs[:])
                    for i in range(DH):
                        for c in range(NMM):
                            pt2 = kps.tile([P, MM_F], F32, tag="mmd", name="mmd")
                            nc.tensor.matmul(pt2[:], w_dt[k][:, i * P:(i + 1) * P],
                                             xdbl_d[:, c * MM_F:(c + 1) * MM_F],
                                             start=True, stop=True)
                            # softplus(dts + dt_b) = ln(1 + exp(dts + dt_b))
                            ed = kdpool.tile([P, MM_F], F32, tag="ed", name="ed",
                                             bufs=2)
                            nc.scalar.activation(ed[:], pt2[:], AF.Exp,
                                                 bias=dt_bias[k][i][:])
                            nc.scalar.activation(delta[i][:, c * MM_F:(c + 1) * MM_F],
                                                 ed[:], AF.Ln, bias=one_b[:])

                    # ---- Phase 4 prep (uses xs_d while still resident) ----
                    dU = [kpool.tile([P, L], F32, tag=f"dU{i}", name=f"dU{i}")
                          for i in range(DH)]
                    state = [kpool.tile([P, DST], F32, tag=f"st{i}",
                                        name=f"st{i}") for i in range(DH)]
                    if k == 1:
                        y_sb = [y1pool.tile([P, L], F32, tag=f"y1_{i}",
                                            name=f"y1_{i}") for i in range(DH)]
                        y1_tiles.extend(y_sb)
                    else:
                        y_sb = [kpool.tile([P, L], F32, tag=f"y0_{i}",
                                           name=f"y0_{i}") for i in range(DH)]
                    for i in range(DH):
                        nc.vector.tensor_tensor(dU[i][:], delta[i][:], xs_d[i][:],
                                                ALU.mult)
                        # y init: Ds * u (skip-connection); scan adds from PSUM
                        nc.vector.tensor_scalar_mul(y_sb[i][:], xs_d[i][:],
                                                    ds_vec[k][i][:])
                        nc.vector.memset(state[i][:], 0.0)

                # ---- Phase 4: selective scan (k -> s -> n -> dh) ----
                with tc.tile_pool(name=f"sc{k}", bufs=2) as work, \
                     tc.tile_pool(name=f"scps{k}", bufs=2,
                                  space=bass.MemorySpace.PSUM) as scps:
                    for s in range(NSLAB):
                        sl = slice(s * LC, (s + 1) * LC)
                        y_ps = [scps.tile([P, LC], F32, tag=f"yps{i}",
                                          name=f"yps{i}") for i in range(DH)]
                        for n in range(DST):
                            brep = work.tile([P, LC], F32, tag="brep", name="brep")
                            nc.sync.dma_start(
                                brep[:],
                                d["bc_dram"][k, 0, n:n + 1, sl].partition_broadcast(P))
                            crep = work.tile([P, LC], F32, tag="crep", name="crep")
                            nc.sync.dma_start(
                                crep[:],
                                d["bc_dram"][k, 1, n:n + 1, sl].partition_broadcast(P))
                            for i in range(DH):
                                dA = work.tile([P, LC], F32, tag=f"dA{i}",
                                               name=f"dA{i}")
                                nc.scalar.activation(dA[:], delta[i][:, sl], AF.Exp,
                                                     scale=a_mat[k][i][:, n:n + 1])
                                xin = work.tile([P, LC], F32, tag="xin",
                                                name="xin")
                                xin_eng = nc.gpsimd if n >= 9 else nc.vector
                                xin_eng.tensor_tensor(xin[:], dU[i][:, sl], brep[:],
                                                      ALU.mult)
                                h = work.tile([P, LC], F32, tag=f"h{i}",
                                              name=f"h{i}")
                                nc.vector.tensor_tensor_scan(
                                    h[:], dA[:], xin[:], state[i][:, n:n + 1],
                                    ALU.mult, ALU.add)
                                nc.vector.tensor_copy(state[i][:, n:n + 1],
                                                      h[:, LC - 1:LC])
                                tmp = work.tile([P, LC], F32, tag="tmp",
                                                name="tmp")
                                nc.gpsimd.tensor_tensor(tmp[:], crep[:], h[:],
                                                        ALU.mult)
                                for hb in range(LC // MM_F):
                                    ps_ = slice(hb * MM_F, (hb + 1) * MM_F)
                                    nc.tensor.matmul(y_ps[i][:, ps_], ident[:],
                                                     tmp[:, ps_],
                                                     start=(n == 0),
                                                     stop=(n == DST - 1))
                        for i in range(DH):
                            nc.vector.scalar_tensor_tensor(
                                y_sb[i][:, sl], y_ps[i][:], 1.0, y_sb[i][:, sl],
                                ALU.mult, ALU.add)
                if k == 0:
                    for i in range(DH):
                        nc.sync.dma_start(d["y0_dram"][i], y_sb[i][:])

        # ================= Phase 5-7: combine, LN, gate, out_proj =================
        with tc.tile_pool(name="fin", bufs=1) as fin:
            y1 = y1_tiles
            ysum = [fin.tile([P, L], F32, tag=f"ys{i}", name=f"ys{i}")
                    for i in range(DH)]
            for i in range(DH):
                y0n = fin.tile([P, L], F32, tag="y0n", name="y0n")
                nc.sync.dma_start(y0n[:], d["y0_dram"][i])
                # y = reorder(y_fwd) + flip(y_rvs), in (t, hw) natural order
                src0 = y0n[:].rearrange("p (hw t) -> p t hw", hw=H * W, t=T)
                src1 = y1[i][:, ::-1].rearrange("p (t hw) -> p t hw", t=T, hw=H * W)
                dst = ysum[i][:].rearrange("p (t hw) -> p t hw", t=T, hw=H * W)
                nc.vector.tensor_tensor(dst, src0, src1, ALU.add)

            # LN stats over DIN (partition reduce via PE ones-contraction)
            stat_mu = fin.tile([1, L], F32, tag="stat_mu", name="stat_mu")
            stat_b = fin.tile([1, L], F32, tag="stat_b", name="stat_b")
            stat_r = fin.tile([1, L], F32, tag="stat_r", name="stat_r")
            with tc.tile_pool(name="fps1", bufs=4,
                              space=bass.MemorySpace.PSUM) as fps1:
                for c in range(NMM):
                    cs = slice(c * MM_F, (c + 1) * MM_F)
                    pmu = fps1.tile([1, MM_F], F32, tag="pmu", name="pmu")
                    nc.tensor.matmul(pmu[:], ones_col[:], ysum[0][:, cs],
                                     start=True, stop=False)
                    nc.tensor.matmul(pmu[:], ones_col[:], ysum[1][:, cs],
                                     start=False, stop=True)
                    nc.scalar.activation(stat_mu[:, cs], pmu[:], AF.Copy)
                    psq = fps1.tile([1, MM_F], F32, tag="psq", name="psq")
                    for i in range(DH):
                        ysq = fin.tile([P, MM_F], F32, tag="ysq", name="ysq")
                        nc.scalar.activation(ysq[:], ysum[i][:, cs], AF.Square)
                        nc.tensor.matmul(psq[:], ones_col[:], ysq[:],
                                         start=(i == 0), stop=(i == DH - 1))
                    nc.scalar.activation(stat_b[:, cs], psq[:], AF.Copy)
            nc.vector.tensor_scalar_mul(stat_mu[:], stat_mu[:], 1.0 / DIN)
            nc.vector.tensor_tensor(stat_r[:], stat_mu[:], stat_mu[:], ALU.mult)
            nc.vector.scalar_tensor_tensor(stat_b[:], stat_b[:], 1.0 / DIN,
                                           stat_r[:], ALU.mult, ALU.subtract)
            eps = fin.tile([1, 1], F32, tag="eps", name="eps")
            nc.vector.memset(eps[:], 1e-5)
            nc.scalar.activation(stat_r[:], stat_b[:], AF.Sqrt, bias=eps[:])
            nc.vector.reciprocal(stat_b[:], stat_r[:])
            mu, rstd = stat_mu, stat_b

            # normalize + affine + gate + out_proj, chunked over L
            with tc.tile_pool(name="fch", bufs=2) as fch, \
                 tc.tile_pool(name="fps2", bufs=2,
                              space=bass.MemorySpace.PSUM) as fps2:
                for c in range(NMM):
                    cs = slice(c * MM_F, (c + 1) * MM_F)
                    murep = fps2.tile([P, MM_F], F32, tag="murep", name="murep")
                    nc.tensor.matmul(murep[:], ones_row[:], mu[:, cs],
                                     start=True, stop=True)
                    rrep = fps2.tile([P, MM_F], F32, tag="rrep", name="rrep")
                    nc.tensor.matmul(rrep[:], ones_row[:], rstd[:, cs],
                                     start=True, stop=True)
                    g = []
                    for i in range(DH):
                        yc = fch.tile([P, MM_F], F32, tag="yc", name="yc")
                        nc.vector.tensor_tensor(yc[:], ysum[i][:, cs], murep[:],
                                                ALU.subtract)
                        yn = fch.tile([P, MM_F], F32, tag="yn", name="yn")
                        nc.vector.tensor_tensor(yn[:], yc[:], rrep[:], ALU.mult)
                        ya = fch.tile([P, MM_F], F32, tag="ya", name="ya")
                        nc.scalar.activation(ya[:], yn[:], AF.Identity,
                                             bias=lnb[i][:], scale=lnw[i][:])
                        zc = fch.tile([P, MM_F], F32, tag=f"zc{i}", name=f"zc{i}")
                        nc.sync.dma_start(zc[:], d["z_dram"][i, :, cs])
                        gi = fch.tile([P, MM_F], F32, tag=f"g{i}", name=f"g{i}")
                        nc.vector.tensor_tensor(gi[:], ya[:], zc[:], ALU.mult)
                        g.append(gi)
                    po = fps2.tile([P, MM_F], F32, tag="pout", name="pout")
                    nc.tensor.matmul(po[:], w_out[0][:], g[0][:],
                                     start=True, stop=False)
                    nc.tensor.matmul(po[:], w_out[1][:], g[1][:],
                                     start=False, stop=True)
                    osb = fch.tile([P, MM_F], F32, tag="osb", name="osb")
                    nc.scalar.activation(osb[:], po[:], AF.Copy)
                    nc.sync.dma_start(d["outT"][:, cs], osb[:])


_CACHE = {}


def _get_program():
    if "nc" not in _CACHE:
        nc = bacc.Bacc("TRN2", target_bir_lowering=False, debug=False,
                       num_devices=NCORES)
        d = _declare_drams(nc)
        with tile.TileContext(nc) as tc:
            _body(tc, d)
        nc.compile()
        _CACHE["nc"] = nc
    return _CACHE["nc"]


def _host_weights(inputs):
    f = lambda a: np.ascontiguousarray(np.asarray(a, np.float32))
    in_proj_w = f(inputs["in_proj_w"])          # (512, 128)
    x_proj_w = f(inputs["x_proj_w"])            # (2, 40, 256)
    dt_w = f(inputs["dt_w"])                    # (2, 256, 8)
    dt_b = f(inputs["dt_b"])                    # (2, 256)
    A_logs = f(inputs["A_logs"])                # (512, 16)
    Ds = f(inputs["Ds"])                        # (512,)
    m = {
        "w_in": f(in_proj_w.T),                                     # (128, 512)
        "conv_sc": f(inputs["conv_w"]).reshape(DH, P, 1),
        "conv_bi": f(inputs["conv_b"]).reshape(DH, P, 1),
        "w_xproj": f(x_proj_w.transpose(0, 2, 1).reshape(KG, DH, P, 40)),
        "w_dt": f(dt_w.transpose(0, 2, 1)),                         # (2, 8, 256)
        "dt_bias": f(dt_b).reshape(KG, DH, P, 1),
        "a_mat": f(-np.exp(A_logs)).reshape(KG, DH, P, DST),
        "ds_vec": f(Ds).reshape(KG, DH, P, 1),
        "lnw": f(inputs["ln_w"]).reshape(DH, P, 1),
        "lnb": f(inputs["ln_b"]).reshape(DH, P, 1),
        "w_out": f(f(inputs["out_proj_w"]).T.reshape(DH, P, P)),
    }
    m["ident"] = np.eye(P, dtype=np.float32)
    return m


def kernel(**inputs):
    x = np.ascontiguousarray(np.asarray(inputs["x"], np.float32))   # (8,16,16,16,128)
    shared = _host_weights(inputs)
    nc = _get_program()
    in_maps = []
    for b in range(NCORES):
        m = dict(shared)
        m["xT"] = np.ascontiguousarray(x[b].reshape(L, DIM).T)
        in_maps.append(m)
    trace = bool(int(os.environ.get("BASS_PROFILE", "0")))
    res = run_bass_kernel_spmd(nc, in_maps, list(range(NCORES)), trace=trace)
    _CACHE["last_result"] = res
    outs = [r["outT"] for r in res.results]
    out = np.stack([o.T.reshape(T, H, W, DIM) for o in outs]).astype(np.float32)
    return out



# revision 4
# speedup vs baseline: 2.1447x; 2.1447x over previous
"""Trainium2 Bass kernel for nn_Block_26628797235524 (Mamba-style cross-scan SSM block).

Sharding: batch B=8 -> one batch element per NeuronCore (SPMD, no collectives).

v2 design (vs v1 baseline at ~2.2ms):
  - bf16 dataflow everywhere precision allows (matmuls, elementwise, scan
    inputs/outputs); the scan's dA decay factor stays fp32 (tensor_tensor_scan
    keeps fp32 internal state, and exp(A*delta)~1 would be wrecked by bf16).
  - Silu activation function on the Scalar engine (1 op instead of
    sigmoid+2 vector multiplies).
  - Full-L (4096) scans: no slab chaining, no state copies.
  - Skip connection Ds*xs injected as a diag(Ds) matmul directly into the
    PSUM y accumulator (start=True), freeing the vector engine.
  - xs orderings are APs (strided / reversed views of xh), never materialized.
  - z and B/C rows spilled to DRAM in bf16; B/C broadcast via DMA
    partition_broadcast per state-dim n.
  - All elementwise scan work on DVE (Pool's bf16 reads are slow and it
    shares SBUF ports with DVE); Scalar does all exp/silu/softplus/copies.
"""

import os
import sys
from contextlib import ExitStack

import numpy as np
import ml_dtypes

_RL = "/opt/trn_rl_repo"
if os.path.isdir(_RL) and _RL not in sys.path:
    sys.path.insert(0, _RL)

import concourse.bass as bass
import concourse.bacc as bacc
import concourse.tile as tile
from concourse import mybir
from concourse.bass_utils import run_bass_kernel_spmd

B, T, H, W, DIM = 8, 16, 16, 16, 128
DIN, DST, DTR, KG = 256, 16, 8, 2
L = T * H * W          # 4096
P = 128
HWC = H * W            # 256
DH = DIN // P          # 2
NCORES = 8
MM_F = 512             # PSUM bank: max matmul free dim (fp32 out)
NMM = L // MM_F        # 8

F32 = mybir.dt.float32
BF16 = mybir.dt.bfloat16
AF = mybir.ActivationFunctionType
ALU = mybir.AluOpType


def _declare_drams(nc):
    d = {}

    def inp(name, shape, dt=BF16):
        d[name] = nc.dram_tensor(name, list(shape), dt, kind="ExternalInput")

    inp("xTb", (P, L))                      # per-core x, channel-major, bf16
    inp("w_in", (P, 4 * P))                 # in_proj_w.T
    inp("conv_sc", (DH, P, 1), F32)
    inp("conv_bi", (DH, P, 1), F32)
    inp("w_xproj", (KG, DH, P, 40))         # x_proj_w[k].T per d-half
    inp("w_dt", (KG, DTR, DIN))             # dt_w[k].T
    inp("dt_bias", (KG, DH, P, 1), F32)
    inp("a_mat", (KG, DH, P, DST), F32)     # A = -exp(A_logs)
    inp("diag_ds", (KG, DH, P, P))          # diag(Ds) per (k, d-half)
    inp("ident", (P, P))
    inp("ones_col", (P, 1))
    inp("one_f32", (P, 1), F32)
    inp("lnw", (DH, P, 1), F32)
    inp("lnb", (DH, P, 1), F32)
    inp("w_out", (DH, P, P))                # out_proj_w.T per d-half
    inp("inv_din", (1, 1), F32)             # 1/256
    inp("neg_one", (1, 1), F32)
    inp("eps11", (1, 1), F32)
    d["z_dram"] = nc.dram_tensor("z_dram", [DH, P, L], BF16)
    d["bc_dram"] = nc.dram_tensor("bc_dram", [KG, 2 * DST, L], BF16)
    d["mr_dram"] = nc.dram_tensor("mr_dram", [2, L], BF16)
    d["outT"] = nc.dram_tensor("outT", [P, L], F32, kind="ExternalOutput")
    return d


def _body(tc, d):
    nc = tc.nc
    with ExitStack() as ctx:
        const = ctx.enter_context(tc.tile_pool(name="const", bufs=1))

        def cload(name, shape, dt=BF16, src=None):
            t = const.tile(list(shape), dt, tag=name, name=name)
            nc.sync.dma_start(t[:], src if src is not None else d[name][:])
            return t

        w_in = cload("w_in", (P, 4 * P))
        conv_sc = [cload(f"conv_sc{i}", (P, 1), F32, d["conv_sc"][i]) for i in range(DH)]
        conv_bi = [cload(f"conv_bi{i}", (P, 1), F32, d["conv_bi"][i]) for i in range(DH)]
        w_xproj = [[cload(f"w_xproj{k}{i}", (P, 40), BF16, d["w_xproj"][k, i])
                    for i in range(DH)] for k in range(KG)]
        w_dt = [cload(f"w_dt{k}", (DTR, DIN), BF16, d["w_dt"][k]) for k in range(KG)]
        dt_bias = [[cload(f"dt_bias{k}{i}", (P, 1), F32, d["dt_bias"][k, i])
                    for i in range(DH)] for k in range(KG)]
        a_mat = [[cload(f"a_mat{k}{i}", (P, DST), F32, d["a_mat"][k, i])
                  for i in range(DH)] for k in range(KG)]
        diag_ds = [[cload(f"diag_ds{k}{i}", (P, P), BF16, d["diag_ds"][k, i])
                    for i in range(DH)] for k in range(KG)]
        ident = cload("ident", (P, P))
        ones_col = cload("ones_col", (P, 1))
        one_f32 = cload("one_f32", (P, 1), F32)
        lnw = [cload(f"lnw{i}", (P, 1), F32, d["lnw"][i]) for i in range(DH)]
        lnb = [cload(f"lnb{i}", (P, 1), F32, d["lnb"][i]) for i in range(DH)]
        w_out = [cload(f"w_out{i}", (P, P), BF16, d["w_out"][i]) for i in range(DH)]
        inv_din = cload("inv_din", (1, 1), F32)
        neg_one = cload("neg_one", (1, 1), F32)
        eps11 = cload("eps11", (1, 1), F32)

        pers = ctx.enter_context(tc.tile_pool(name="pers", bufs=1))
        xh = [pers.tile([P, L], BF16, tag=f"xh{i}", name=f"xh{i}") for i in range(DH)]
        delta = [[pers.tile([P, L], BF16, tag=f"dl{k}{i}", name=f"dl{k}{i}")
                  for i in range(DH)] for k in range(KG)]
        y_sb = [[pers.tile([P, L], BF16, tag=f"y{k}{i}", name=f"y{k}{i}")
                 for i in range(DH)] for k in range(KG)]

        # xs views: k=0 spectral order (hw, t); k=1 reversed natural order.
        def xs_full(i, k):
            if k == 0:
                return xh[i][:].rearrange("p (t hw) -> p hw t", t=T, hw=HWC)
            return xh[i][:, ::-1]

        def xs_chunk(i, k, c):
            if k == 0:
                v = xh[i][:].rearrange("p (t hw) -> p hw t", t=T, hw=HWC)
                nh = MM_F // T  # 32 hw values per 512-col chunk
                return v[:, c * nh:(c + 1) * nh, :]
            hi = L - 1 - c * MM_F
            lo = L - (c + 1) * MM_F - 1
            return xh[i][:, hi:(None if lo < 0 else lo):-1]

        # ================= Phase A: in_proj + depthwise conv + silu ==========
        with tc.tile_pool(name="pA", bufs=1) as pA, \
             tc.tile_pool(name="psA", bufs=4, space=bass.MemorySpace.PSUM) as psA:
            xTb = pA.tile([P, L], BF16, tag="xTb", name="xTb")
            nc.sync.dma_start(xTb[:], d["xTb"][:])
            for po in range(4):
                for c in range(NMM):
                    cs = slice(c * MM_F, (c + 1) * MM_F)
                    ps = psA.tile([P, MM_F], F32, tag="pa", name="pa")
                    nc.tensor.matmul(ps[:], w_in[:, po * P:(po + 1) * P],
                                     xTb[:, cs], start=True, stop=True)
                    if po < DH:
                        nc.scalar.activation(xh[po][:, cs], ps[:], AF.Silu,
                                             bias=conv_bi[po][:], scale=conv_sc[po][:])
                    else:
                        zc = pA.tile([P, MM_F], BF16, tag="zc", name="zc", bufs=3)
                        nc.scalar.activation(zc[:], ps[:], AF.Silu)
                        nc.sync.dma_start(d["z_dram"][po - DH, :, cs], zc[:])

        # ================= Phase B: x_dbl -> B/C spill, delta ================
        with tc.tile_pool(name="pB", bufs=1) as pB, \
             tc.tile_pool(name="psB", bufs=2, space=bass.MemorySpace.PSUM) as psB:
            for k in range(KG):
                xdbl = pB.tile([40, L], BF16, tag="xdbl", name=f"xdbl{k}", bufs=2)
                for c in range(NMM):
                    cs = slice(c * MM_F, (c + 1) * MM_F)
                    ps = psB.tile([40, MM_F], F32, tag="pb", name="pb")
                    nc.tensor.matmul(ps[:], w_xproj[k][0][:], xs_chunk(0, k, c),
                                     start=True, stop=False)
                    nc.tensor.matmul(ps[:], w_xproj[k][1][:], xs_chunk(1, k, c),
                                     start=False, stop=True)
                    nc.scalar.activation(xdbl[:, cs], ps[:], AF.Copy)
                nc.sync.dma_start(d["bc_dram"][k], xdbl[DTR:40, :])
                for i in range(DH):
                    for c in range(NMM):
                        cs = slice(c * MM_F, (c + 1) * MM_F)
                        ps2 = psB.tile([P, MM_F], F32, tag="pb2", name="pb2")
                        nc.tensor.matmul(ps2[:], w_dt[k][:, i * P:(i + 1) * P],
                                         xdbl[0:DTR, cs], start=True, stop=True)
                        # softplus(x + dt_b) = ln(1 + exp(x + dt_b))
                        ed = pB.tile([P, MM_F], F32, tag="ed", name="ed", bufs=2)
                        nc.scalar.activation(ed[:], ps2[:], AF.Exp,
                                             bias=dt_bias[k][i][:])
                        nc.scalar.activation(delta[k][i][:, cs], ed[:], AF.Ln,
                                             bias=one_f32[:])

        # ================= Phase C: selective scan ===========================
        with tc.tile_pool(name="sc", bufs=2) as sc, \
             tc.tile_pool(name="psC", bufs=1, space=bass.MemorySpace.PSUM) as psC:
            for k in range(KG):
                for i in range(DH):
                    dU = sc.tile([P, L], BF16, tag="dU", name="dU", bufs=1)
                    if k == 0:
                        nc.vector.tensor_tensor(
                            dU[:].rearrange("p (hw t) -> p hw t", hw=HWC, t=T),
                            delta[k][i][:].rearrange("p (hw t) -> p hw t", hw=HWC, t=T),
                            xs_full(i, k), ALU.mult)
                    else:
                        nc.vector.tensor_tensor(dU[:], delta[k][i][:],
                                                xs_full(i, k), ALU.mult)
                    y_ps = psC.tile([P, L], F32, tag="yps", name="yps")
                    for c in range(NMM):
                        nc.tensor.matmul(y_ps[:, c * MM_F:(c + 1) * MM_F],
                                         diag_ds[k][i][:], xs_chunk(i, k, c),
                                         start=True, stop=False)
                    for n in range(DST):
                        brep = sc.tile([P, L], BF16, tag="brep", name="brep")
                        nc.sync.dma_start(
                            brep[:], d["bc_dram"][k, n:n + 1, :].partition_broadcast(P))
                        crep = sc.tile([P, L], BF16, tag="crep", name="crep")
                        nc.sync.dma_start(
                            crep[:],
                            d["bc_dram"][k, DST + n:DST + n + 1, :].partition_broadcast(P))
                        dA = sc.tile([P, L], F32, tag="dA", name="dA")
                        nc.scalar.activation(dA[:], delta[k][i][:], AF.Exp,
                                             scale=a_mat[k][i][:, n:n + 1])
                        xin = sc.tile([P, L], BF16, tag="xin", name="xin")
                        nc.vector.tensor_tensor(xin[:], dU[:], brep[:], ALU.mult)
                        h = sc.tile([P, L], BF16, tag="h", name="h")
                        nc.vector.tensor_tensor_scan(h[:], dA[:], xin[:], 0.0,
                                                     ALU.mult, ALU.add)
                        tmp = sc.tile([P, L], BF16, tag="tmp", name="tmp")
                        nc.vector.tensor_tensor(tmp[:], crep[:], h[:], ALU.mult)
                        for c in range(NMM):
                            cs = slice(c * MM_F, (c + 1) * MM_F)
                            nc.tensor.matmul(y_ps[:, cs], ident[:], tmp[:, cs],
                                             start=False, stop=(n == DST - 1))
                    nc.scalar.activation(y_sb[k][i][:], y_ps[:], AF.Copy)

        # ================= Phase D: combine + LN + gate + out_proj ===========
        with tc.tile_pool(name="pD", bufs=1) as pD, \
             tc.tile_pool(name="psD", bufs=2, space=bass.MemorySpace.PSUM) as psD:
            ysum = [pD.tile([P, L], BF16, tag=f"ys{i}", name=f"ys{i}")
                    for i in range(DH)]
            for i in range(DH):
                y0v = y_sb[0][i][:].rearrange("p (hw t) -> p t hw", hw=HWC, t=T)
                y1v = y_sb[1][i][:, ::-1].rearrange("p (t hw) -> p t hw", t=T, hw=HWC)
                dst = ysum[i][:].rearrange("p (t hw) -> p t hw", t=T, hw=HWC)
                nc.vector.tensor_tensor(dst, y0v, y1v, ALU.add)

            for c in range(NMM):
                cs = slice(c * MM_F, (c + 1) * MM_F)
                ps1 = psD.tile([1, MM_F], F32, tag="ps1", name="ps1")
                nc.tensor.matmul(ps1[:], ones_col[:], ysum[0][:, cs],
                                 start=True, stop=False)
                nc.tensor.matmul(ps1[:], ones_col[:], ysum[1][:, cs],
                                 start=False, stop=True)
                ps2 = psD.tile([1, MM_F], F32, tag="ps2", name="ps2")
                for i in range(DH):
                    yq = pD.tile([P, MM_F], BF16, tag="yq", name="yq", bufs=2)
                    nc.scalar.activation(yq[:], ysum[i][:, cs], AF.Square)
                    nc.tensor.matmul(ps2[:], ones_col[:], yq[:],
                                     start=(i == 0), stop=(i == DH - 1))
                mu = pD.tile([1, MM_F], F32, tag="mu", name="mu", bufs=2)
                nc.scalar.activation(mu[:], ps1[:], AF.Identity, scale=inv_din[:])
                e2 = pD.tile([1, MM_F], F32, tag="e2", name="e2", bufs=2)
                nc.scalar.activation(e2[:], ps2[:], AF.Identity, scale=inv_din[:])
                m2 = pD.tile([1, MM_F], F32, tag="m2", name="m2", bufs=2)
                nc.scalar.activation(m2[:], mu[:], AF.Square)
                var = pD.tile([1, MM_F], F32, tag="var", name="var", bufs=2)
                nc.vector.tensor_tensor(var[:], e2[:], m2[:], ALU.subtract)
                sd = pD.tile([1, MM_F], F32, tag="sd", name="sd", bufs=2)
                nc.scalar.activation(sd[:], var[:], AF.Sqrt, bias=eps11[:])
                rr = pD.tile([1, MM_F], F32, tag="rr", name="rr", bufs=2)
                nc.vector.reciprocal(rr[:], sd[:])
                a_row = pD.tile([1, MM_F], BF16, tag="a_row", name="a_row", bufs=2)
                nc.scalar.activation(a_row[:], rr[:], AF.Copy)
                t1 = pD.tile([1, MM_F], F32, tag="t1", name="t1", bufs=2)
                nc.vector.tensor_tensor(t1[:], mu[:], rr[:], ALU.mult)
                b_row = pD.tile([1, MM_F], BF16, tag="b_row", name="b_row", bufs=2)
                nc.scalar.activation(b_row[:], t1[:], AF.Identity, scale=neg_one[:])
                nc.sync.dma_start(d["mr_dram"][0:1, cs], a_row[:])
                nc.sync.dma_start(d["mr_dram"][1:2, cs], b_row[:])
            arep = pD.tile([P, L], BF16, tag="arep", name="arep")
            nc.sync.dma_start(arep[:], d["mr_dram"][0:1, :].partition_broadcast(P))
            brep_ln = pD.tile([P, L], BF16, tag="brepl", name="brepl")
            nc.sync.dma_start(brep_ln[:], d["mr_dram"][1:2, :].partition_broadcast(P))

            for c in range(NMM):
                cs = slice(c * MM_F, (c + 1) * MM_F)
                out_ps = psD.tile([P, MM_F], F32, tag="ops", name="ops")
                for i in range(DH):
                    zc = pD.tile([P, MM_F], BF16, tag="zc2", name="zc2", bufs=3)
                    nc.sync.dma_start(zc[:], d["z_dram"][i, :, cs])
                    yn = pD.tile([P, MM_F], BF16, tag="yn", name="yn", bufs=2)
                    nc.vector.tensor_tensor(yn[:], ysum[i][:, cs], arep[:, cs],
                                            ALU.mult)
                    yn2 = pD.tile([P, MM_F], BF16, tag="yn2", name="yn2", bufs=2)
                    nc.vector.tensor_tensor(yn2[:], yn[:], brep_ln[:, cs], ALU.add)
                    ya = pD.tile([P, MM_F], BF16, tag="ya", name="ya", bufs=2)
                    nc.scalar.activation(ya[:], yn2[:], AF.Identity,
                                         bias=lnb[i][:], scale=lnw[i][:])
                    g = pD.tile([P, MM_F], BF16, tag="g", name="g", bufs=2)
                    nc.vector.tensor_tensor(g[:], ya[:], zc[:], ALU.mult)
                    nc.tensor.matmul(out_ps[:], w_out[i][:], g[:],
                                     start=(i == 0), stop=(i == DH - 1))
                osb = pD.tile([P, MM_F], F32, tag="osb", name="osb", bufs=2)
                nc.scalar.activation(osb[:], out_ps[:], AF.Copy)
                nc.sync.dma_start(d["outT"][:, cs], osb[:])


_CACHE = {}


def _get_program():
    if "nc" not in _CACHE:
        nc = bacc.Bacc("TRN2", target_bir_lowering=False, debug=False,
                       num_devices=NCORES)
        d = _declare_drams(nc)
        with tile.TileContext(nc) as tc:
            _body(tc, d)
        nc.compile()
        _CACHE["nc"] = nc
    return _CACHE["nc"]


def _host_weights(inputs):
    f32 = lambda a: np.ascontiguousarray(np.asarray(a, np.float32))
    bf = lambda a: np.ascontiguousarray(np.asarray(a, np.float32)).astype(ml_dtypes.bfloat16)
    in_proj_w = f32(inputs["in_proj_w"])            # (512, 128)
    x_proj_w = f32(inputs["x_proj_w"])              # (2, 40, 256)
    dt_w = f32(inputs["dt_w"])                      # (2, 256, 8)
    dt_b = f32(inputs["dt_b"])                      # (2, 256)
    A_logs = f32(inputs["A_logs"])                  # (512, 16)
    Ds = f32(inputs["Ds"])                          # (512,)
    diag_ds = np.zeros((KG, DH, P, P), np.float32)
    for k in range(KG):
        for i in range(DH):
            np.fill_diagonal(diag_ds[k, i], Ds[k * DIN + i * P:k * DIN + (i + 1) * P])
    m = {
        "w_in": bf(in_proj_w.T),
        "conv_sc": f32(inputs["conv_w"]).reshape(DH, P, 1),
        "conv_bi": f32(inputs["conv_b"]).reshape(DH, P, 1),
        "w_xproj": bf(x_proj_w.transpose(0, 2, 1).reshape(KG, DH, P, 40)),
        "w_dt": bf(dt_w.transpose(0, 2, 1)),
        "dt_bias": f32(dt_b).reshape(KG, DH, P, 1),
        "a_mat": f32(-np.exp(A_logs)).reshape(KG, DH, P, DST),
        "diag_ds": diag_ds.astype(ml_dtypes.bfloat16),
        "ident": np.eye(P, dtype=np.float32).astype(ml_dtypes.bfloat16),
        "ones_col": np.ones((P, 1), np.float32).astype(ml_dtypes.bfloat16),
        "one_f32": np.ones((P, 1), np.float32),
        "lnw": f32(inputs["ln_w"]).reshape(DH, P, 1),
        "lnb": f32(inputs["ln_b"]).reshape(DH, P, 1),
        "w_out": bf(f32(inputs["out_proj_w"]).T.reshape(DH, P, P)),
        "inv_din": np.full((1, 1), 1.0 / DIN, np.float32),
        "neg_one": np.full((1, 1), -1.0, np.float32),
        "eps11": np.full((1, 1), 1e-5, np.float32),
    }
    return m


def kernel(**inputs):
    x = np.ascontiguousarray(np.asarray(inputs["x"], np.float32))   # (8,16,16,16,128)
    shared = _host_weights(inputs)
    nc = _get_program()
    in_maps = []
    for b in range(NCORES):
        m = dict(shared)
        m["xTb"] = np.ascontiguousarray(
            x[b].reshape(L, DIM).T).astype(ml_dtypes.bfloat16)
        in_maps.append(m)
    trace = bool(int(os.environ.get("BASS_PROFILE", "0")))
    res = run_bass_kernel_spmd(nc, in_maps, list(range(NCORES)), trace=trace)
    _CACHE["last_result"] = res
    outs = [np.asarray(r["outT"], np.float32) for r in res.results]
    out = np.stack([o.T.reshape(T, H, W, DIM) for o in outs]).astype(np.float32)
    return out


# revision 11
# speedup vs baseline: 2.2112x; 1.0310x over previous
"""Trainium2 Bass kernel for nn_Block_26628797235524 (Mamba-style cross-scan SSM block).

Sharding: batch B=8 -> one batch element per NeuronCore (SPMD, no collectives).

v2 design (vs v1 baseline at ~2.2ms):
  - bf16 dataflow everywhere precision allows (matmuls, elementwise, scan
    inputs/outputs); the scan's dA decay factor stays fp32 (tensor_tensor_scan
    keeps fp32 internal state, and exp(A*delta)~1 would be wrecked by bf16).
  - Silu activation function on the Scalar engine (1 op instead of
    sigmoid+2 vector multiplies).
  - Full-L (4096) scans: no slab chaining, no state copies.
  - Skip connection Ds*xs injected as a diag(Ds) matmul directly into the
    PSUM y accumulator (start=True), freeing the vector engine.
  - xs orderings are APs (strided / reversed views of xh), never materialized.
  - z and B/C rows spilled to DRAM in bf16; B/C broadcast via DMA
    partition_broadcast per state-dim n.
  - All elementwise scan work on DVE (Pool's bf16 reads are slow and it
    shares SBUF ports with DVE); Scalar does all exp/silu/softplus/copies.
"""

import os
import sys
from contextlib import ExitStack

import numpy as np
import ml_dtypes

_RL = "/opt/trn_rl_repo"
if os.path.isdir(_RL) and _RL not in sys.path:
    sys.path.insert(0, _RL)

import concourse.bass as bass
import concourse.bacc as bacc
import concourse.tile as tile
from concourse import mybir
from concourse.bass_utils import run_bass_kernel_spmd

B, T, H, W, DIM = 8, 16, 16, 16, 128
DIN, DST, DTR, KG = 256, 16, 8, 2
L = T * H * W          # 4096
P = 128
HWC = H * W            # 256
DH = DIN // P          # 2
NCORES = 8
MM_F = 512             # PSUM bank: max matmul free dim (fp32 out)
NMM = L // MM_F        # 8

F32 = mybir.dt.float32
BF16 = mybir.dt.bfloat16
AF = mybir.ActivationFunctionType
ALU = mybir.AluOpType


def _declare_drams(nc):
    d = {}

    def inp(name, shape, dt=BF16):
        d[name] = nc.dram_tensor(name, list(shape), dt, kind="ExternalInput")

    inp("xTb", (P, L))                      # per-core x, channel-major, bf16
    inp("w_in", (P, 4 * P))                 # in_proj_w.T
    inp("conv_sc", (DH, P, 1), F32)
    inp("conv_bi", (DH, P, 1), F32)
    inp("w_xproj", (KG, DH, P, 40))         # x_proj_w[k].T per d-half
    inp("w_dt", (KG, DTR, DIN))             # dt_w[k].T
    inp("dt_bias", (KG, DH, P, 1), F32)
    inp("a_mat", (KG, DH, P, DST), F32)     # A = -exp(A_logs)
    inp("diag_ds", (KG, DH, P, P))          # diag(Ds) per (k, d-half)
    inp("ident", (P, P))
    inp("ones_col", (P, 1))
    inp("one_f32", (P, 1), F32)
    inp("lnw", (DH, P, 1), F32)
    inp("lnb", (DH, P, 1), F32)
    inp("w_out", (DH, P, P))                # out_proj_w.T per d-half
    inp("inv_din", (1, 1), F32)             # 1/256
    inp("neg_one", (1, 1), F32)
    inp("eps11", (1, 1), F32)
    d["z_dram"] = nc.dram_tensor("z_dram", [DH, P, L], BF16)
    d["bc_dram"] = nc.dram_tensor("bc_dram", [KG, 2 * DST, L], BF16)
    d["mr_dram"] = nc.dram_tensor("mr_dram", [2, L], BF16)
    d["outT"] = nc.dram_tensor("outT", [P, L], F32, kind="ExternalOutput")
    return d


def _body(tc, d):
    nc = tc.nc
    with ExitStack() as ctx:
        const = ctx.enter_context(tc.tile_pool(name="const", bufs=1))

        def cload(name, shape, dt=BF16, src=None):
            t = const.tile(list(shape), dt, tag=name, name=name)
            nc.sync.dma_start(t[:], src if src is not None else d[name][:])
            return t

        w_in = cload("w_in", (P, 4 * P))
        conv_sc = [cload(f"conv_sc{i}", (P, 1), F32, d["conv_sc"][i]) for i in range(DH)]
        conv_bi = [cload(f"conv_bi{i}", (P, 1), F32, d["conv_bi"][i]) for i in range(DH)]
        w_xproj = [[cload(f"w_xproj{k}{i}", (P, 40), BF16, d["w_xproj"][k, i])
                    for i in range(DH)] for k in range(KG)]
        w_dt = [cload(f"w_dt{k}", (DTR, DIN), BF16, d["w_dt"][k]) for k in range(KG)]
        dt_bias = [[cload(f"dt_bias{k}{i}", (P, 1), F32, d["dt_bias"][k, i])
                    for i in range(DH)] for k in range(KG)]
        a_mat = [[cload(f"a_mat{k}{i}", (P, DST), F32, d["a_mat"][k, i])
                  for i in range(DH)] for k in range(KG)]
        diag_ds = [[cload(f"diag_ds{k}{i}", (P, P), BF16, d["diag_ds"][k, i])
                    for i in range(DH)] for k in range(KG)]
        ident = cload("ident", (P, P))
        ones_col = cload("ones_col", (P, 1))
        one_f32 = cload("one_f32", (P, 1), F32)
        lnw = [cload(f"lnw{i}", (P, 1), F32, d["lnw"][i]) for i in range(DH)]
        lnb = [cload(f"lnb{i}", (P, 1), F32, d["lnb"][i]) for i in range(DH)]
        w_out = [cload(f"w_out{i}", (P, P), BF16, d["w_out"][i]) for i in range(DH)]
        inv_din = cload("inv_din", (1, 1), F32)
        neg_one = cload("neg_one", (1, 1), F32)
        eps11 = cload("eps11", (1, 1), F32)

        pers = ctx.enter_context(tc.tile_pool(name="pers", bufs=1))
        xh = [pers.tile([P, L], BF16, tag=f"xh{i}", name=f"xh{i}") for i in range(DH)]
        delta = [[pers.tile([P, L], BF16, tag=f"dl{k}{i}", name=f"dl{k}{i}")
                  for i in range(DH)] for k in range(KG)]
        y_sb = [[pers.tile([P, L], BF16, tag=f"y{k}{i}", name=f"y{k}{i}")
                 for i in range(DH)] for k in range(KG)]

        # xs views: k=0 spectral order (hw, t); k=1 reversed natural order.
        def xs_full(i, k):
            if k == 0:
                return xh[i][:].rearrange("p (t hw) -> p hw t", t=T, hw=HWC)
            return xh[i][:, ::-1]

        def xs_chunk(i, k, c):
            if k == 0:
                v = xh[i][:].rearrange("p (t hw) -> p hw t", t=T, hw=HWC)
                nh = MM_F // T  # 32 hw values per 512-col chunk
                return v[:, c * nh:(c + 1) * nh, :]
            hi = L - 1 - c * MM_F
            lo = L - (c + 1) * MM_F - 1
            return xh[i][:, hi:(None if lo < 0 else lo):-1]

        # ================= Phase A: in_proj + depthwise conv + silu ==========
        with tc.tile_pool(name="pA", bufs=1) as pA, \
             tc.tile_pool(name="psA", bufs=4, space=bass.MemorySpace.PSUM) as psA:
            xTb = pA.tile([P, L], BF16, tag="xTb", name="xTb")
            nc.sync.dma_start(xTb[:], d["xTb"][:])
            for po in range(4):
                for c in range(NMM):
                    cs = slice(c * MM_F, (c + 1) * MM_F)
                    ps = psA.tile([P, MM_F], F32, tag="pa", name="pa")
                    nc.tensor.matmul(ps[:], w_in[:, po * P:(po + 1) * P],
                                     xTb[:, cs], start=True, stop=True)
                    if po < DH:
                        nc.scalar.activation(xh[po][:, cs], ps[:], AF.Silu,
                                             bias=conv_bi[po][:], scale=conv_sc[po][:])
                    else:
                        zc = pA.tile([P, MM_F], BF16, tag="zc", name="zc", bufs=3)
                        nc.scalar.activation(zc[:], ps[:], AF.Silu)
                        nc.sync.dma_start(d["z_dram"][po - DH, :, cs], zc[:])

        # ================= Phase B: x_dbl -> B/C spill, delta ================
        with tc.tile_pool(name="pB", bufs=1) as pB, \
             tc.tile_pool(name="psB", bufs=2, space=bass.MemorySpace.PSUM) as psB:
            for k in range(KG):
                xdbl = pB.tile([40, L], BF16, tag="xdbl", name=f"xdbl{k}", bufs=2)
                for c in range(NMM):
                    cs = slice(c * MM_F, (c + 1) * MM_F)
                    ps = psB.tile([40, MM_F], F32, tag="pb", name="pb")
                    nc.tensor.matmul(ps[:], w_xproj[k][0][:], xs_chunk(0, k, c),
                                     start=True, stop=False)
                    nc.tensor.matmul(ps[:], w_xproj[k][1][:], xs_chunk(1, k, c),
                                     start=False, stop=True)
                    nc.scalar.activation(xdbl[:, cs], ps[:], AF.Copy)
                nc.sync.dma_start(d["bc_dram"][k], xdbl[DTR:40, :])
                for i in range(DH):
                    # softplus(x + dt_b) = ln(1 + exp(x + dt_b)); batch the 8
                    # Exp chunks then one full-width Ln so the Exp/Ln act
                    # tables load once each instead of alternating.
                    ed = pB.tile([P, L], F32, tag="ed", name="ed", bufs=1)
                    for c in range(NMM):
                        cs = slice(c * MM_F, (c + 1) * MM_F)
                        ps2 = psB.tile([P, MM_F], F32, tag="pb2", name="pb2")
                        nc.tensor.matmul(ps2[:], w_dt[k][:, i * P:(i + 1) * P],
                                         xdbl[0:DTR, cs], start=True, stop=True)
                        nc.scalar.activation(ed[:, cs], ps2[:], AF.Exp,
                                             bias=dt_bias[k][i][:])
                    nc.scalar.activation(delta[k][i][:], ed[:], AF.Ln,
                                         bias=one_f32[:])

        # ================= Phase C: selective scan ===========================
        # Half-L slabs: smaller tiles allow 4-deep broadcast prefetch (the
        # full-L version stalled the scan ~10us/iter on brep/crep DMAs), and
        # the two d-halves interleave per n. State chains across halves via a
        # first-column fixup (xin[0] += dA[0]*state) so `initial` stays 0.0.
        LC = L // 2
        NMC = LC // MM_F
        with tc.tile_pool(name="sc", bufs=2) as sc, \
             tc.tile_pool(name="psC", bufs=1, space=bass.MemorySpace.PSUM) as psC:
            state = sc.tile([P, 2 * DST], F32, tag="state", name="state", bufs=1)
            for k in range(KG):
                dUs = []
                for i in range(DH):
                    dU = sc.tile([P, L], BF16, tag=f"dU{i}", name=f"dU{i}", bufs=1)
                    if k == 0:
                        nc.vector.tensor_tensor(
                            dU[:].rearrange("p (hw t) -> p hw t", hw=HWC, t=T),
                            delta[k][i][:].rearrange("p (hw t) -> p hw t", hw=HWC, t=T),
                            xs_full(i, k), ALU.mult)
                    else:
                        nc.vector.tensor_tensor(dU[:], delta[k][i][:],
                                                xs_full(i, k), ALU.mult)
                    dUs.append(dU)
                for half in range(2):
                    hs = slice(half * LC, (half + 1) * LC)
                    y_ps = [psC.tile([P, LC], F32, tag=f"yps{i}", name=f"yps{i}")
                            for i in range(DH)]
                    for i in range(DH):
                        for c in range(NMC):
                            nc.tensor.matmul(
                                y_ps[i][:, c * MM_F:(c + 1) * MM_F],
                                diag_ds[k][i][:],
                                xs_chunk(i, k, half * NMC + c),
                                start=True, stop=False)
                    for n in range(DST):
                        brep = sc.tile([P, LC], BF16, tag="brep", name="brep",
                                       bufs=4)
                        nc.sync.dma_start(
                            brep[:],
                            d["bc_dram"][k, n:n + 1, hs].partition_broadcast(P))
                        crep = sc.tile([P, LC], BF16, tag="crep", name="crep",
                                       bufs=4)
                        nc.sync.dma_start(
                            crep[:],
                            d["bc_dram"][k, DST + n:DST + n + 1,
                                         hs].partition_broadcast(P))
                        for i in range(DH):
                            col = 2 * n + i
                            dA = sc.tile([P, LC], F32, tag="dA", name="dA")
                            nc.scalar.activation(dA[:], delta[k][i][:, hs],
                                                 AF.Exp,
                                                 scale=a_mat[k][i][:, n:n + 1])
                            xin = sc.tile([P, LC], BF16, tag="xin", name="xin")
                            nc.vector.tensor_tensor(xin[:], dUs[i][:, hs],
                                                    brep[:], ALU.mult)
                            if half == 1:
                                nc.vector.scalar_tensor_tensor(
                                    xin[:, 0:1], dA[:, 0:1],
                                    state[:, col:col + 1], xin[:, 0:1],
                                    ALU.mult, ALU.add)
                            h = sc.tile([P, LC], BF16, tag="h", name="h")
                            nc.vector.tensor_tensor_scan(h[:], dA[:], xin[:],
                                                         0.0, ALU.mult, ALU.add)
                            if half == 0:
                                nc.vector.tensor_copy(state[:, col:col + 1],
                                                      h[:, LC - 1:LC])
                            tmp = sc.tile([P, LC], BF16, tag="tmp", name="tmp")
                            nc.vector.tensor_tensor(tmp[:], crep[:], h[:],
                                                    ALU.mult)
                            for c in range(NMC):
                                cs = slice(c * MM_F, (c + 1) * MM_F)
                                nc.tensor.matmul(y_ps[i][:, cs], ident[:],
                                                 tmp[:, cs], start=False,
                                                 stop=(n == DST - 1))
                    for i in range(DH):
                        nc.scalar.activation(y_sb[k][i][:, hs], y_ps[i][:],
                                             AF.Copy)

        # ================= Phase D: combine + LN + gate + out_proj ===========
        with tc.tile_pool(name="pD", bufs=1) as pD, \
             tc.tile_pool(name="psD", bufs=2, space=bass.MemorySpace.PSUM) as psD:
            ysum = [pD.tile([P, L], BF16, tag=f"ys{i}", name=f"ys{i}")
                    for i in range(DH)]
            for i in range(DH):
                y0v = y_sb[0][i][:].rearrange("p (hw t) -> p t hw", hw=HWC, t=T)
                y1v = y_sb[1][i][:, ::-1].rearrange("p (t hw) -> p t hw", t=T, hw=HWC)
                dst = ysum[i][:].rearrange("p (t hw) -> p t hw", t=T, hw=HWC)
                nc.vector.tensor_tensor(dst, y0v, y1v, ALU.add)

            for c in range(NMM):
                cs = slice(c * MM_F, (c + 1) * MM_F)
                ps1 = psD.tile([1, MM_F], F32, tag="ps1", name="ps1")
                nc.tensor.matmul(ps1[:], ones_col[:], ysum[0][:, cs],
                                 start=True, stop=False)
                nc.tensor.matmul(ps1[:], ones_col[:], ysum[1][:, cs],
                                 start=False, stop=True)
                ps2 = psD.tile([1, MM_F], F32, tag="ps2", name="ps2")
                for i in range(DH):
                    yq = pD.tile([P, MM_F], BF16, tag="yq", name="yq", bufs=2)
                    nc.scalar.activation(yq[:], ysum[i][:, cs], AF.Square)
                    nc.tensor.matmul(ps2[:], ones_col[:], yq[:],
                                     start=(i == 0), stop=(i == DH - 1))
                mu = pD.tile([1, MM_F], F32, tag="mu", name="mu", bufs=2)
                nc.scalar.activation(mu[:], ps1[:], AF.Identity, scale=inv_din[:])
                e2 = pD.tile([1, MM_F], F32, tag="e2", name="e2", bufs=2)
                nc.scalar.activation(e2[:], ps2[:], AF.Identity, scale=inv_din[:])
                m2 = pD.tile([1, MM_F], F32, tag="m2", name="m2", bufs=2)
                nc.scalar.activation(m2[:], mu[:], AF.Square)
                var = pD.tile([1, MM_F], F32, tag="var", name="var", bufs=2)
                nc.vector.tensor_tensor(var[:], e2[:], m2[:], ALU.subtract)
                sd = pD.tile([1, MM_F], F32, tag="sd", name="sd", bufs=2)
                nc.scalar.activation(sd[:], var[:], AF.Sqrt, bias=eps11[:])
                rr = pD.tile([1, MM_F], F32, tag="rr", name="rr", bufs=2)
                nc.vector.reciprocal_approx_fast(rr[:], sd[:])
                a_row = pD.tile([1, MM_F], BF16, tag="a_row", name="a_row", bufs=2)
                nc.scalar.activation(a_row[:], rr[:], AF.Copy)
                t1 = pD.tile([1, MM_F], F32, tag="t1", name="t1", bufs=2)
                nc.vector.tensor_tensor(t1[:], mu[:], rr[:], ALU.mult)
                b_row = pD.tile([1, MM_F], BF16, tag="b_row", name="b_row", bufs=2)
                nc.scalar.activation(b_row[:], t1[:], AF.Identity, scale=neg_one[:])
                nc.sync.dma_start(d["mr_dram"][0:1, cs], a_row[:])
                nc.sync.dma_start(d["mr_dram"][1:2, cs], b_row[:])
            arep = pD.tile([P, L], BF16, tag="arep", name="arep")
            nc.sync.dma_start(arep[:], d["mr_dram"][0:1, :].partition_broadcast(P))
            brep_ln = pD.tile([P, L], BF16, tag="brepl", name="brepl")
            nc.sync.dma_start(brep_ln[:], d["mr_dram"][1:2, :].partition_broadcast(P))

            for c in range(NMM):
                cs = slice(c * MM_F, (c + 1) * MM_F)
                out_ps = psD.tile([P, MM_F], F32, tag="ops", name="ops")
                for i in range(DH):
                    zc = pD.tile([P, MM_F], BF16, tag="zc2", name="zc2", bufs=3)
                    nc.sync.dma_start(zc[:], d["z_dram"][i, :, cs])
                    yn = pD.tile([P, MM_F], BF16, tag="yn", name="yn", bufs=2)
                    nc.vector.tensor_tensor(yn[:], ysum[i][:, cs], arep[:, cs],
                                            ALU.mult)
                    yn2 = pD.tile([P, MM_F], BF16, tag="yn2", name="yn2", bufs=2)
                    nc.vector.tensor_tensor(yn2[:], yn[:], brep_ln[:, cs], ALU.add)
                    ya = pD.tile([P, MM_F], BF16, tag="ya", name="ya", bufs=2)
                    nc.scalar.activation(ya[:], yn2[:], AF.Identity,
                                         bias=lnb[i][:], scale=lnw[i][:])
                    g = pD.tile([P, MM_F], BF16, tag="g", name="g", bufs=2)
                    nc.vector.tensor_tensor(g[:], ya[:], zc[:], ALU.mult)
                    nc.tensor.matmul(out_ps[:], w_out[i][:], g[:],
                                     start=(i == 0), stop=(i == DH - 1))
                osb = pD.tile([P, MM_F], F32, tag="osb", name="osb", bufs=2)
                nc.scalar.activation(osb[:], out_ps[:], AF.Copy)
                nc.sync.dma_start(d["outT"][:, cs], osb[:])


_CACHE = {}


def _get_program():
    if "nc" not in _CACHE:
        nc = bacc.Bacc("TRN2", target_bir_lowering=False, debug=False,
                       num_devices=NCORES)
        d = _declare_drams(nc)
        with tile.TileContext(nc) as tc:
            _body(tc, d)
        nc.compile()
        _CACHE["nc"] = nc
    return _CACHE["nc"]


def _host_weights(inputs):
    f32 = lambda a: np.ascontiguousarray(np.asarray(a, np.float32))
    bf = lambda a: np.ascontiguousarray(np.asarray(a, np.float32)).astype(ml_dtypes.bfloat16)
    in_proj_w = f32(inputs["in_proj_w"])            # (512, 128)
    x_proj_w = f32(inputs["x_proj_w"])              # (2, 40, 256)
    dt_w = f32(inputs["dt_w"])                      # (2, 256, 8)
    dt_b = f32(inputs["dt_b"])                      # (2, 256)
    A_logs = f32(inputs["A_logs"])                  # (512, 16)
    Ds = f32(inputs["Ds"])                          # (512,)
    diag_ds = np.zeros((KG, DH, P, P), np.float32)
    for k in range(KG):
        for i in range(DH):
            np.fill_diagonal(diag_ds[k, i], Ds[k * DIN + i * P:k * DIN + (i + 1) * P])
    m = {
        "w_in": bf(in_proj_w.T),
        "conv_sc": f32(inputs["conv_w"]).reshape(DH, P, 1),
        "conv_bi": f32(inputs["conv_b"]).reshape(DH, P, 1),
        "w_xproj": bf(x_proj_w.transpose(0, 2, 1).reshape(KG, DH, P, 40)),
        "w_dt": bf(dt_w.transpose(0, 2, 1)),
        "dt_bias": f32(dt_b).reshape(KG, DH, P, 1),
        "a_mat": f32(-np.exp(A_logs)).reshape(KG, DH, P, DST),
        "diag_ds": diag_ds.astype(ml_dtypes.bfloat16),
        "ident": np.eye(P, dtype=np.float32).astype(ml_dtypes.bfloat16),
        "ones_col": np.ones((P, 1), np.float32).astype(ml_dtypes.bfloat16),
        "one_f32": np.ones((P, 1), np.float32),
        "lnw": f32(inputs["ln_w"]).reshape(DH, P, 1),
        "lnb": f32(inputs["ln_b"]).reshape(DH, P, 1),
        "w_out": bf(f32(inputs["out_proj_w"]).T.reshape(DH, P, P)),
        "inv_din": np.full((1, 1), 1.0 / DIN, np.float32),
        "neg_one": np.full((1, 1), -1.0, np.float32),
        "eps11": np.full((1, 1), 1e-5, np.float32),
    }
    return m


def kernel(**inputs):
    x = np.ascontiguousarray(np.asarray(inputs["x"], np.float32))   # (8,16,16,16,128)
    shared = _host_weights(inputs)
    nc = _get_program()
    in_maps = []
    for b in range(NCORES):
        m = dict(shared)
        m["xTb"] = np.ascontiguousarray(
            x[b].reshape(L, DIM).T).astype(ml_dtypes.bfloat16)
        in_maps.append(m)
    trace = bool(int(os.environ.get("BASS_PROFILE", "0")))
    res = run_bass_kernel_spmd(nc, in_maps, list(range(NCORES)), trace=trace)
    _CACHE["last_result"] = res
    outs = [np.asarray(r["outT"], np.float32) for r in res.results]
    out = np.stack([o.T.reshape(T, H, W, DIM) for o in outs]).astype(np.float32)
    return out
